# revision 1
# baseline (speedup 1.0000x reference)
"""Trainium2 Bass kernel for nn_CustomDynamicEdgeConv.

Problem (per graph, B=64 graphs of NPG=1024 nodes, F=64, H=128, K=16):
  1. knn_graph (k=16, no self loops) within each graph by euclidean distance.
     edge (row=r, col=c): r in top-16 nearest of center c.
  2. ef = [x[r], x[c]-x[r]]  -> Linear(128->128) -> ReLU -> Linear(128->128) -> ReLU
  3. scatter-mean of edge features onto r; counts clamped to >= 1.

Sharding: 8 graphs per NeuronCore (data parallel over B), weights replicated.

Per-core pipeline (all shapes per graph g):
  Phase A: load x tiles [128,64], compute sq=|x|^2 (ACT square+accum),
           build aug tile [128,66] = [x, -sq/2, 1], PE-transpose ->
           xt2 [66,1024] f32 (metric operand), xtb [64,1024] bf16 (MM1a rhs),
           x2tab bf16 gather table (node-major, 256B rows = [x_bf16, zeros]).
  Phase B: metric s = -d/2 = x_i.x_j - sq_i/2 - sq_j/2 via ONE fp32 matmul
           with K=66 (xt2 as both lhsT chunk and rhs; the sq/ones rows fold
           both -sq/2 terms).  Diagonal killed with a -1e10*I matmul (bf16).
           Top-16 per row directly in PSUM: max8 / max_index / match_replace
           / max8 / max_index (DVE). ids -> +g*1024 -> f32 -> PE-transpose ->
           int16 "wrapped" index layout [16,1024]; DRAM bounce replicates to
           [128,1024] for the gpsimd DMA index ops.
  Phase C: per 2048-edge group: dma_gather (SBUF-src transpose mode) pulls
           x2tab columns feature-major [128,2048] bf16 (rows 64..127 zero);
           MM1a (W1b x_c, rhs = xtb with repeat-16 AP) + MM1b ([W1a-W1b;0],
           gathered) accumulate h1pre in PSUM; ACT relu+b1 -> h1 bf16;
           MM2 per 128-edge tile with h1 as stationary and [W2] moving after
           a b2 prefill matmul (ones^T b2row) -> h2pre edge-major in PSUM;
           ACT relu -> staging [128,16,129] bf16 (col 128 = 1.0 edge count);
           gpsimd dma_scatter_add accumulates rows of [h2,1] into the DRAM
           accumulator acc[8192,256] bf16 (elem 129, row stride 256).
  Phase D: out = acc[:,0:128] / max(acc[:,128],1).

kernel(**inputs) takes FULL inputs and returns the FULL [65536,128] f32 output.
"""

import numpy as np

NPG = 1024
F = 64
H = 128
K = 16
NCORES = 8

_CACHE = {}


def _build(G):
    """Build the per-core Bass program for G graphs (NN = G*1024 nodes)."""
    import concourse.bass as bass
    import concourse.bacc as bacc
    import concourse.mybir as mybir
    from concourse.tile import TileContext
    from concourse import library_config
    from contextlib import ExitStack

    dt = mybir.dt
    AF = mybir.ActivationFunctionType
    NN = G * NPG
    NEG = -1.0e10

    nc = bacc.Bacc("TRN2", target_bir_lowering=False, debug=False, num_devices=1)

    x_in = nc.dram_tensor("x", [NN, F], dt.float32, kind="ExternalInput")
    w1b_d = nc.dram_tensor("w1b", [F, H], dt.bfloat16, kind="ExternalInput")
    w1p_d = nc.dram_tensor("w1pf", [F, H], dt.float32, kind="ExternalInput")
    w2_d = nc.dram_tensor("w2", [H, H], dt.bfloat16, kind="ExternalInput")
    b1_d = nc.dram_tensor("b1", [H, 1], dt.float32, kind="ExternalInput")
    b2_d = nc.dram_tensor("b2row", [1, H], dt.bfloat16, kind="ExternalInput")
    on_d = nc.dram_tensor("ones1", [1, H], dt.bfloat16, kind="ExternalInput")
    id_d = nc.dram_tensor("identf", [128, 128], dt.float32, kind="ExternalInput")
    eye_d = nc.dram_tensor("eyeb", [128, 128], dt.bfloat16, kind="ExternalInput")
    neye_d = nc.dram_tensor("negeyeb", [128, 128], dt.bfloat16, kind="ExternalInput")
    out_d = nc.dram_tensor("out", [NN, H], dt.float32, kind="ExternalOutput")

    J = 32  # expansion factor: scatter row = r*J + (c mod J); zero dup targets per call
    exp_d = [nc.dram_tensor(f"exp{i}", [J * NPG, 256], dt.bfloat16) for i in range(2)]
    wib_d = nc.dram_tensor("widx_bounce", [G, 16, NPG], dt.int16)
    spat_d = nc.dram_tensor("spat", [128, NPG], dt.int16, kind="ExternalInput")

    with TileContext(nc) as tc, ExitStack() as ctx:
        nc.gpsimd.load_library(library_config.mlp)
        cpool = ctx.enter_context(tc.tile_pool(name="consts", bufs=1))
        # ---- persistent constants ----
        cw1b = cpool.tile([F, H], dt.bfloat16)
        nc.sync.dma_start(cw1b[:], w1b_d[:])
        cw1p = cpool.tile([F, H], dt.float32)
        nc.sync.dma_start(cw1p[:], w1p_d[:])
        cw2 = cpool.tile([H, H], dt.bfloat16)
        nc.sync.dma_start(cw2[:], w2_d[:])
        cb1 = cpool.tile([H, 1], dt.float32)
        nc.sync.dma_start(cb1[:], b1_d[:])
        cb2 = cpool.tile([1, H], dt.bfloat16)
        nc.sync.dma_start(cb2[:], b2_d[:])
        cone = cpool.tile([1, H], dt.bfloat16)
        nc.sync.dma_start(cone[:], on_d[:])
        cid = cpool.tile([128, 128], dt.float32)
        nc.sync.dma_start(cid[:], id_d[:])
        ceye = cpool.tile([128, 128], dt.bfloat16)
        nc.sync.dma_start(ceye[:], eye_d[:])
        cneye = cpool.tile([128, 128], dt.bfloat16)
        nc.sync.dma_start(cneye[:], neye_d[:])
        cspat = cpool.tile([128, NPG], dt.int16)
        nc.sync.dma_start(cspat[:], spat_d[:])

        # feature-major bf16 x table for ap_gather: [64, NN]
        xtball = cpool.tile([F, NN], dt.bfloat16)
        # zero tile used to clear the expanded scatter tables
        zt = cpool.tile([128, 129], dt.bfloat16)
        nc.vector.memset(zt[:], 0.0)

        # ---- pools ----
        xaug_p = ctx.enter_context(tc.tile_pool(name="xaug", bufs=4))
        sdump_p = ctx.enter_context(tc.tile_pool(name="sdump", bufs=3))
        pst_p = ctx.enter_context(tc.tile_pool(name="pst", bufs=1, space="PSUM"))
        xt2_p = ctx.enter_context(tc.tile_pool(name="xt2", bufs=2))
        xlh_p = ctx.enter_context(tc.tile_pool(name="xlh", bufs=2))
        ps_s_p = ctx.enter_context(tc.tile_pool(name="ps_s", bufs=2, space="PSUM"))
        ssb_p = ctx.enter_context(tc.tile_pool(name="ssb", bufs=3))
        v8_p = ctx.enter_context(tc.tile_pool(name="v8", bufs=4))
        ids_p = ctx.enter_context(tc.tile_pool(name="ids", bufs=4))
        idsf_p = ctx.enter_context(tc.tile_pool(name="idsf", bufs=4))

        widx16_p = ctx.enter_context(tc.tile_pool(name="widx16", bufs=3))
        widx_p = ctx.enter_context(tc.tile_pool(name="widx", bufs=3))
        xg_p = ctx.enter_context(tc.tile_pool(name="xg", bufs=4))
        ps_mlp_p = ctx.enter_context(tc.tile_pool(name="ps_mlp", bufs=3, space="PSUM"))
        h1_p = ctx.enter_context(tc.tile_pool(name="h1", bufs=3))

        stag_p = ctx.enter_context(tc.tile_pool(name="stag", bufs=3))
        fin_p = ctx.enter_context(tc.tile_pool(name="fin", bufs=4))

        for g in range(G):
            # ---------------- Phase A ----------------
            xt2 = xt2_p.tile([66, NPG], dt.float32)
            xlh = xlh_p.tile([65, NPG], dt.float32)
            nc.vector.memset(xlh[64:65, :], 1.0)
            for t in range(8):
                xaug = xaug_p.tile([128, 66], dt.float32)
                nc.sync.dma_start(
                    xaug[:, 0:F], x_in[g * NPG + t * 128 : g * NPG + (t + 1) * 128, :]
                )
                sdump = sdump_p.tile([128, F], dt.float32)
                sq = sdump_p.tile([128, 1], dt.float32, tag="sq")
                nc.scalar.activation(
                    sdump[:], xaug[:, 0:F], AF.Square, accum_out=sq[:]
                )
                nc.vector.tensor_scalar(
                    xaug[:, 64:65], sq[:], -0.5, None, mybir.AluOpType.mult
                )
                nc.vector.memset(xaug[:, 65:66], 1.0)
                pst = pst_p.tile([128, 128], dt.float32, tag="pst")
                nc.tensor.transpose(pst[0:66, :], xaug[:], cid[:])
                nc.scalar.activation(
                    xt2[0:65, t * 128 : (t + 1) * 128], pst[0:65, :], AF.Copy
                )
                nc.vector.tensor_copy(
                    xlh[0:F, t * 128 : (t + 1) * 128], pst[0:F, :]
                )
                nc.vector.tensor_copy(
                    xtball[:, g * NPG + t * 128 : g * NPG + (t + 1) * 128], pst[0:F, :]
                )

            # ---------------- Phase B ----------------
            widx16 = widx16_p.tile([16, NPG], dt.int16)
            for t in range(8):
                ps_s = ps_s_p.tile([128, NPG], dt.float32)
                hh = t // 4  # which 512-half holds the diagonal block
                for h in range(2):
                    nc.tensor.matmul(
                        ps_s[:, h * 512 : (h + 1) * 512],
                        xlh[:, t * 128 : (t + 1) * 128],
                        xt2[0:65, h * 512 : (h + 1) * 512],
                        start=True,
                        stop=(h != hh),
                        skip_group_check=True,
                    )
                nc.tensor.matmul(
                    ps_s[:, t * 128 : (t + 1) * 128],
                    cneye[:],
                    ceye[:],
                    start=False,
                    stop=True,
                    skip_group_check=True,
                )
                # stage s out of PSUM so TensorE isn't blocked on the
                # 5-pass DVE selection; ACT does the copy.
                s_sb = ssb_p.tile([128, NPG], dt.float32)
                nc.scalar.activation(s_sb[:], ps_s[:], AF.Copy)
                # top-16 (self already pushed to -1e10 by the diag matmul)
                v1 = v8_p.tile([128, 8], dt.float32, tag="v1")
                v2 = v8_p.tile([128, 8], dt.float32, tag="v2")
                ids = ids_p.tile([128, 16], dt.uint16)
                nc.vector.max(v1[:], s_sb[:])
                nc.vector.max_index(ids[:, 0:8], v1[:], s_sb[:])
                nc.vector.match_replace(s_sb[:], v1[:], s_sb[:], NEG)
                nc.vector.max(v2[:], s_sb[:])
                nc.vector.max_index(ids[:, 8:16], v2[:], s_sb[:])
                idsf = idsf_p.tile([128, 16], dt.float32)
                nc.vector.tensor_scalar(
                    idsf[:], ids[:], 0.0, None, mybir.AluOpType.add
                )
                psit = pst_p.tile([128, 128], dt.float32, tag="pst")
                nc.tensor.transpose(psit[0:16, :], idsf[:], cid[:])
                nc.vector.tensor_copy(
                    widx16[:, t * 128 : (t + 1) * 128], psit[0:16, :]
                )
            nc.sync.dma_start(wib_d[g], widx16[:])
            widx = widx_p.tile([128, NPG], dt.int16)
            nc.sync.dma_start(
                widx[:], wib_d[g].unsqueeze(0).broadcast_to([8, 16, NPG])
            )
            widxE = widx_p.tile([128, NPG], dt.int16, tag="widxE")
            nc.vector.scalar_tensor_tensor(
                widxE[:], widx[:], float(J), cspat[:],
                mybir.AluOpType.mult, mybir.AluOpType.add,
            )
            # zero this graph's expansion table (cols 0:129 only)
            eg = exp_d[g % 2]
            nc.sync.dma_start(
                eg.rearrange("(a p) c -> p a c", p=128)[:, :, 0:129],
                zt[:].unsqueeze(1).broadcast_to([128, J * NPG // 128, 129]),
            )

            # ---------------- Phase C ----------------
            nc.gpsimd.load_library(library_config.ap_gather)
            xgs = []
            for grp in range(8):
                xg = xg_p.tile([F, 2048], dt.float32)
                nc.gpsimd.ap_gather(
                    xg[:].unsqueeze(2),
                    xt2[0:F, :].unsqueeze(2),
                    widx[0:F, grp * 128 : (grp + 1) * 128],
                    channels=F,
                    num_elems=NPG,
                    d=1,
                    num_idxs=2048,
                )
                xgs.append(xg)
            nc.gpsimd.load_library(library_config.mlp)
            for grp in range(8):
                xg = xgs[grp]
                h1 = h1_p.tile([128, 2048], dt.bfloat16)
                for sub in range(4):
                    cs0 = grp * 128 + sub * 32
                    ps_h1 = ps_mlp_p.tile([128, 512], dt.float32, tag="mlp")
                    nc.tensor.matmul(
                        ps_h1[:],
                        cw1b[:],
                        xtball[:, g * NPG + cs0 : g * NPG + cs0 + 32]
                        .unsqueeze(2)
                        .broadcast_to([F, 32, 16]),
                        start=True,
                        stop=False,
                    )
                    nc.tensor.matmul(
                        ps_h1[:],
                        cw1p[:],
                        xg[:, sub * 512 : (sub + 1) * 512],
                        start=False,
                        stop=True,
                    )
                    nc.scalar.activation(
                        h1[:, sub * 512 : (sub + 1) * 512],
                        ps_h1[:],
                        AF.Relu,
                        bias=cb1[:],
                    )
                stag = stag_p.tile([128, 16, 129], dt.bfloat16)
                nc.vector.memset(stag[:, :, 128:129], 1.0)
                for sub in range(4):
                    ps_h2 = ps_mlp_p.tile([128, 512], dt.float32, tag="mlp")
                    nc.tensor.matmul(
                        ps_h2[:],
                        cone[:],
                        cb2[:].unsqueeze(1).broadcast_to([1, 4, 128]),
                        start=True,
                        stop=False,
                        skip_group_check=True,
                    )
                    for tt in range(4):
                        T = sub * 4 + tt
                        nc.tensor.matmul(
                            ps_h2[:, tt * 128 : (tt + 1) * 128],
                            h1[:, T * 128 : (T + 1) * 128],
                            cw2[:],
                            start=False,
                            stop=True,
                            skip_group_check=True,
                        )
                    nc.scalar.activation(
                        stag[:, sub * 4 : (sub + 1) * 4, 0:128],
                        ps_h2[:],
                        AF.Relu,
                    )
                for ci in range(4):
                    nc.gpsimd.dma_scatter_add(
                        eg[:, 0:129],
                        stag[:, 4 * ci : 4 * ci + 4, :],
                        widxE[:, grp * 128 + 32 * ci : grp * 128 + 32 * (ci + 1)],
                        num_idxs=512,
                        num_idxs_reg=512,
                        elem_size=129,
                        elem_step=256,
                    )

            # ---------------- Phase D (per graph) ----------------
            for i in range(8):
                acb = fin_p.tile([128, J, 129], dt.bfloat16)
                nc.sync.dma_start(
                    acb[:],
                    eg[i * 128 * J : (i + 1) * 128 * J, 0:129].rearrange(
                        "(r q) c -> r q c", q=J
                    ),
                )
                r16 = fin_p.tile([128, 16, 129], dt.bfloat16, tag="r16")
                nc.vector.tensor_tensor(
                    r16[:], acb[:, 0:16, :], acb[:, 16:32, :], mybir.AluOpType.add
                )
                r8 = fin_p.tile([128, 8, 129], dt.bfloat16, tag="r8")
                nc.vector.tensor_tensor(
                    r8[:], r16[:, 0:8, :], r16[:, 8:16, :], mybir.AluOpType.add
                )
                r4 = fin_p.tile([128, 4, 129], dt.bfloat16, tag="r4")
                nc.vector.tensor_tensor(
                    r4[:], r8[:, 0:4, :], r8[:, 4:8, :], mybir.AluOpType.add
                )
                r2 = fin_p.tile([128, 2, 129], dt.bfloat16, tag="r2")
                nc.vector.tensor_tensor(
                    r2[:], r4[:, 0:2, :], r4[:, 2:4, :], mybir.AluOpType.add
                )
                red = fin_p.tile([128, 129], dt.float32, tag="red")
                nc.vector.tensor_tensor(
                    red[:], r2[:, 0, :], r2[:, 1, :], mybir.AluOpType.add
                )
                cnt = fin_p.tile([128, 1], dt.float32, tag="cnt")
                nc.vector.tensor_scalar_max(cnt[:], red[:, 128:129], 1.0)
                rec = fin_p.tile([128, 1], dt.float32, tag="rec")
                nc.vector.reciprocal(rec[:], cnt[:])
                ot = fin_p.tile([128, H], dt.float32, tag="ot")
                nc.vector.tensor_scalar(
                    ot[:], red[:, 0:H], rec[:], None, mybir.AluOpType.mult
                )
                nc.sync.dma_start(
                    out_d[g * NPG + i * 128 : g * NPG + (i + 1) * 128, :], ot[:]
                )

    nc.compile()
    return nc


def _consts(W1, b1, W2, b2):
    import ml_dtypes

    bf = ml_dtypes.bfloat16
    W1 = np.asarray(W1, np.float32)
    W2 = np.asarray(W2, np.float32)
    b1 = np.asarray(b1, np.float32)
    b2 = np.asarray(b2, np.float32)
    w1a, w1b = W1[0:F], W1[F : 2 * F]
    return {
        "w1b": w1b.astype(bf),
        "w1pf": (w1a - w1b).astype(np.float32),
        "w2": W2.astype(bf),
        "b1": b1.reshape(H, 1),
        "b2row": b2.reshape(1, H).astype(bf),
        "ones1": np.ones((1, H), bf),
        "identf": np.eye(128, dtype=np.float32),
        "eyeb": np.eye(128, dtype=np.float32).astype(bf),
        "negeyeb": (np.eye(128, dtype=np.float32) * -1.0e10).astype(bf),
        "spat": np.tile((np.arange(1024, dtype=np.int16) % 32)[None, :], (128, 1)),
    }


def _get_program(G):
    if G not in _CACHE:
        _CACHE[G] = _build(G)
    return _CACHE[G]


def kernel(x, batch, W1, b1, W2, b2):
    from concourse.bass_utils import run_bass_kernel_spmd

    x = np.asarray(x, np.float32)
    G = 64 // NCORES
    NN = G * NPG
    nc = _get_program(G)
    consts = _consts(W1, b1, W2, b2)
    in_maps = [
        {"x": np.ascontiguousarray(x[c * NN : (c + 1) * NN]), **consts}
        for c in range(NCORES)
    ]
    res = run_bass_kernel_spmd(nc, in_maps, list(range(NCORES)))
    out = np.concatenate([res.results[c]["out"] for c in range(NCORES)], axis=0)
    return out.astype(np.float32)

